# revision 2
# baseline (speedup 1.0000x reference)
"""Trainium2 Bass kernel for nn_AttentionManifold (B=32, P=128, IN=64, OUT=32).

Data-parallel over batch: each of 8 NeuronCores handles 4 batches.
Per core:
  A. Q/K/V = W x W^T: shared-stationary f32r matmuls + DVE 32x32 block
     transpose between the two contractions. Kinds (q,k,v) on partition
     strips 0-31/32-63/64-95.
  B. logm via inverse scaling-squaring: 2 scaled Newton-Schulz sqrt stages
     (deferred-scalar form, offline-tuned alphas, global normalizer
     c=8.5 folded into compile-time scalars) + degree-11 log series.
     Per-sample 32x32 matmuls on tile_position diagonal strips.
     log A = 4*p(E) + ln(c) I, with the constant diagonal term added via a
     host-provided (const * I) tile.
  C. attention: Gram via 32 per-j accumulating matmuls in [key, query]
     layout, qq/kk via ones-matmul broadcasts, softmax along free axis.
  D. Frechet mean: mean_logT = LVflat^T @ S^T chunks; expm via
     scaling-squaring (Taylor d=8, 5 squarings) on 4-sample strips.
"""
import math
import numpy as np

import concourse.bacc as bacc
import concourse.mybir as mybir
import concourse.tile as tile
from concourse.bass_utils import run_bass_kernel_spmd

F32 = mybir.dt.float32
F32R = mybir.dt.float32r
MULT = mybir.AluOpType.mult
ADD = mybir.AluOpType.add

B, P, IN = 32, 128, 64
NCORES = 8
BLOC = B // NCORES
GRP = 16
NGRP = P // GRP

CGLOB = 8.5                      # global SPD normalizer, > lambda_max (~7.3)
ALPHAS0 = [1.7939874036898087, 1.6696029929467766, 1.5753856846965621,
           1.3802459084155867, 1.1355312114962206, 1.0145731825395088,
           1.0001600783454123]
ALPHAS1 = [1.639353436157538, 1.3943732234795634, 1.1476361656772485,
           1.0173994934181363, 1.000228417137108]
# log series coeffs on M in [0.1627, 0.9658]: log(M) ~ sum_k SER[k] (M-I)^k
SER = [0.00025761896563381015, 1.016394391935819, -0.08934176002367167,
       5.76267183490063, 42.59363464146395, 215.5576662374658,
       713.4419495013208, 1577.6143678674662, 2302.793898554353,
       2133.2456306970385, 1137.177063455271, 266.64841671372346]
SDEG = len(SER) - 1              # 11
POW2S = 4.0
LNC_CONST = math.log(CGLOB) + POW2S * SER[0]
EXP_K = 5
EXP_D = 8
EXP_C = [1.0 / math.factorial(k) for k in range(EXP_D + 1)]


def _stage_scalars():
    out = []
    for st, alphas in enumerate((ALPHAS0, ALPHAS1)):
        gammas = []
        p = (1.0 / CGLOB) if st == 0 else 1.0
        q = 1.0
        for a in alphas:
            gammas.append(-(a * a / 3.0) * (p * q))
            p *= 1.5 * a
            q *= 1.5 * a
        out.append((gammas, p))
    return out

STAGE_SCALARS = _stage_scalars()


def _persample_round(nc, out_ap_fn, lhs_fn, rhs_fn, nsamp, tile_col_fn=None):
    for w in range(3):
        for s in range(nsamp):
            tc_ = (32 * w, 32 * w) if tile_col_fn is None else (32 * w, tile_col_fn(w))
            nc.tensor.matmul(out_ap_fn(w, s), lhs_fn(w, s), rhs_fn(w, s),
                             start=True, stop=True, tile_position=tc_)


def build_nc(debug_stage=99):
    nc = bacc.Bacc("TRN2", target_bir_lowering=False, debug=False,
                   num_devices=NCORES)
    x_in = nc.dram_tensor("x", [BLOC, P, IN, IN], F32, kind="ExternalInput").ap()
    wallT_in = nc.dram_tensor("wallT", [IN, 96], F32, kind="ExternalInput").ap()
    wall2_in = nc.dram_tensor("wall2", [96, 64], F32, kind="ExternalInput").ap()
    ibig_in = nc.dram_tensor("ibig", [128, GRP, 32], F32, kind="ExternalInput").ap()
    ibigl_in = nc.dram_tensor("ibigl", [32, 3 * GRP * 32], F32, kind="ExternalInput").ap()
    ibgx_in = nc.dram_tensor("ibgx", [128, 32, 32], F32, kind="ExternalInput").ap()
    id128_in = nc.dram_tensor("id128", [128, 128], F32, kind="ExternalInput").ap()
    ones_in = nc.dram_tensor("onesc", [96, 128], F32, kind="ExternalInput").ap()
    out_d = nc.dram_tensor("out", [BLOC, P, 32, 32], F32, kind="ExternalOutput").ap()

    dbg = {}
    def dbg_out(name, shape):
        dbg[name] = nc.dram_tensor(name, shape, F32, kind="ExternalOutput").ap()
    if debug_stage == 1:
        dbg_out("d_mats", [BLOC, 96, P, 32])
    if debug_stage == 2:
        dbg_out("d_mhat", [BLOC, 96, P, 32])
        dbg_out("d_mats", [BLOC, 96, P, 32])
        dbg_out("d_m0", [BLOC, 96, P, 32])
        dbg_out("d_c0", [BLOC, 96, P, 32])
        dbg_out("d_yz1", [BLOC, 96, P, 64])
    if debug_stage == 3:
        dbg_out("d_lf", [BLOC, 32, 3, P, 32])
        dbg_out("d_mats", [BLOC, 96, P, 32])
    if debug_stage in (5, 6, 7):
        dbg_out("d_gt", [BLOC, 128, 32, 32])
    if debug_stage == 5:
        dbg_out("d_tay", [BLOC, 128, 32, 32])
        dbg_out("d_mlfs", [BLOC, 128, 1024])
    if debug_stage == 4:
        dbg_out("d_en", [BLOC, 128, 128])
        dbg_out("d_s", [BLOC, 128, 128])
        dbg_out("d_mlfs", [BLOC, 128, 1024])

    with tile.TileContext(nc) as tc:
        with (
            tc.tile_pool(name="const", bufs=1) as cpool,
            tc.tile_pool(name="perb", bufs=1) as bpool,
            tc.tile_pool(name="grp", bufs=2) as gpool,
            tc.tile_pool(name="ps", bufs=1, space="PSUM") as ps,
            tc.tile_pool(name="dscr", bufs=1, space="DRAM") as dpool,
        ):
            scrV_t = dpool.tile([32, P, 32], F32, name="scrV")
            scrM_t = dpool.tile([P, 1024], F32, name="scrM")
            scrV = scrV_t[:]
            scrM = scrM_t[:]
            wallT = cpool.tile([IN, 96], F32)
            nc.sync.dma_start(wallT[:], wallT_in[:])
            wallTr = cpool.tile([IN, 96], F32R)
            nc.vector.tensor_copy(wallTr[:], wallT[:])
            wall2 = cpool.tile([96, 2, 32], F32)
            nc.sync.dma_start(wall2[:], wall2_in.rearrange("p (h j) -> p h j", h=2))
            ibig = cpool.tile([128, GRP, 32], F32)
            nc.sync.dma_start(ibig[:], ibig_in[:])
            ibigl = cpool.tile([32, 3, GRP, 32], F32)
            nc.sync.dma_start(ibigl[:], ibigl_in.rearrange(
                "p (w s j) -> p w s j", w=3, s=GRP))
            ibgx = cpool.tile([128, 32, 32], F32)
            nc.sync.dma_start(ibgx[:], ibgx_in[:])
            id128 = cpool.tile([128, 128], F32)
            nc.sync.dma_start(id128[:], id128_in[:])
            onesc = cpool.tile([96, 128], F32)
            nc.sync.dma_start(onesc[:], ones_in[:])

            for b in range(BLOC):
                # ================= stage A =================
                xt = bpool.tile([IN, P, IN], F32, tag="xt", bufs=1)
                nc.sync.dma_start(xt[:], x_in[b].rearrange("p i j -> i p j"))
                ytT = bpool.tile([96, P, 2, 32], F32, tag="ytT", bufs=1)
                for t in range(16):
                    xs = xt[:].rearrange("i p j -> i (p j)")[:, 512 * t:512 * (t + 1)]
                    xr = gpool.tile([IN, 512], F32R, tag="xr")
                    nc.vector.tensor_copy(xr[:], xs)
                    psY = ps.tile([96, 512], F32, tag="p1a", name=f"psY{b}_{t}")
                    nc.tensor.matmul(psY[:], wallTr[:], xr[:], start=True, stop=True)
                    nc.vector.transpose(
                        ytT[:].rearrange("p m h j -> p (m h j)")[:, 512 * t:512 * (t + 1)],
                        psY[:])
                mats = bpool.tile([96, P, 32], F32, tag="mats", bufs=1)
                for g in range(NGRP):
                    gsl = slice(GRP * g, GRP * (g + 1))
                    psQ = ps.tile([96, GRP, 32], F32, tag="p1b", name=f"psQ{b}_{g}")
                    for h in range(2):
                        for w in range(3):
                            sl = slice(32 * w, 32 * w + 32)
                            nc.tensor.matmul(
                                psQ[sl, :, :], wall2[sl, h, :],
                                ytT[sl, gsl, h, :],
                                start=(h == 0), stop=(h == 1),
                                tile_position=(32 * w, 32 * w))
                    nc.scalar.copy(mats[:, gsl, :], psQ[:])
                if "d_mats" in dbg:
                    nc.sync.dma_start(dbg["d_mats"][b], mats[:])
                if debug_stage <= 1:
                    continue

                # ================= stage B: logm =================
                lf = (bpool.tile([32, 3, P, 32], F32, tag="lf", bufs=1,
                                 name="lf")
                      if debug_stage != 2 else None)
                mhat_dbg = (bpool.tile([96, P, 32], F32, tag="mhat", bufs=1,
                                        name="mhat_dbg")
                            if "d_mhat" in dbg else None)
                m0_dbg = (bpool.tile([96, P, 32], F32, tag="m0d", bufs=1,
                                     name="m0_dbg")
                          if "d_m0" in dbg else None)
                yz1_dbg = (bpool.tile([96, P, 64], F32, tag="yz1d", bufs=1,
                                      name="yz1_dbg")
                           if "d_yz1" in dbg else None)

                c0_dbg = (bpool.tile([96, P, 32], F32, tag="c0d", bufs=1,
                                     name="c0_dbg")
                          if "d_c0" in dbg else None)
                for g in range(NGRP):
                    gsl = slice(GRP * g, GRP * (g + 1))
                    yz = gpool.tile([96, GRP, 64], F32, tag="yz")
                    mcur = gpool.tile([96, GRP, 32], F32, tag="mcur")
                    nc.vector.tensor_copy(mcur[:], mats[:, gsl, :])
                    ctile = gpool.tile([96, GRP, 32], F32, tag="ctile")
                    for st in range(2):
                        gammas, pn = STAGE_SCALARS[st]
                        nc.vector.scalar_tensor_tensor(
                            ctile[:], mcur[:], float(gammas[0]), ibig[0:96],
                            op0=MULT, op1=ADD)
                        if st == 0 and c0_dbg is not None:
                            nc.vector.tensor_copy(c0_dbg[:, gsl, :], ctile[:])
                        psYZ = ps.tile([96, GRP, 64], F32, tag="p2a",
                                       name=f"psYZ0_{b}_{g}_{st}")
                        _persample_round(
                            nc,
                            lambda w, s: psYZ[32 * w:32 * w + 32, s, 0:32],
                            lambda w, s: ctile[32 * w:32 * w + 32, s, :],
                            lambda w, s: mcur[32 * w:32 * w + 32, s, :], GRP)
                        nc.scalar.copy(yz[:, :, 0:32], psYZ[:, :, 0:32])
                        nc.vector.tensor_copy(yz[:, :, 32:64], ctile[:])
                        for k in range(1, len(gammas)):
                            psT = ps.tile([96, GRP, 32], F32, tag="p1a",
                                          name=f"psT{b}_{g}_{st}_{k}")
                            _persample_round(
                                nc,
                                lambda w, s: psT[32 * w:32 * w + 32, s, :],
                                lambda w, s: yz[32 * w:32 * w + 32, s, 32:64],
                                lambda w, s: yz[32 * w:32 * w + 32, s, 0:32], GRP)
                            nc.vector.scalar_tensor_tensor(
                                ctile[:], psT[:], float(gammas[k]), ibig[0:96],
                                op0=MULT, op1=ADD)
                            psYZ = ps.tile([96, GRP, 64], F32, tag="p2a",
                                           name=f"psYZ{b}_{g}_{st}_{k}")
                            _persample_round(
                                nc,
                                lambda w, s: psYZ[32 * w:32 * w + 32, s, :],
                                lambda w, s: ctile[32 * w:32 * w + 32, s, :],
                                lambda w, s: yz[32 * w:32 * w + 32, s, :], GRP)
                            if k == len(gammas) - 1:
                                nc.scalar.mul(mcur[:], psYZ[:, :, 0:32], float(pn))
                            else:
                                nc.scalar.copy(yz[:], psYZ[:])
                            if st == 0 and k == 1 and yz1_dbg is not None:
                                nc.vector.tensor_copy(yz1_dbg[:, gsl, :], yz[:])

                        if st == 0 and m0_dbg is not None:
                            nc.vector.tensor_copy(m0_dbg[:, gsl, :], mcur[:])
                    if mhat_dbg is not None:
                        nc.vector.tensor_copy(mhat_dbg[:, gsl, :], mcur[:])
                        continue
                    # series (coeffs pre-scaled by 4 = 2^s)
                    etile = gpool.tile([96, GRP, 32], F32, tag="etile")
                    nc.vector.scalar_tensor_tensor(
                        etile[:], ibig[0:96], -1.0, mcur[:], op0=MULT, op1=ADD)
                    acc = gpool.tile([96, GRP, 32], F32, tag="acc")
                    nc.vector.tensor_scalar_mul(acc[:], ibig[0:96],
                                                float(POW2S * SER[SDEG]))
                    for k in range(SDEG - 1, 0, -1):
                        psH = ps.tile([96, GRP, 32], F32, tag="p1a",
                                      name=f"psH{b}_{g}_{k}")
                        _persample_round(
                            nc,
                            lambda w, s: psH[32 * w:32 * w + 32, s, :],
                            lambda w, s: etile[32 * w:32 * w + 32, s, :],
                            lambda w, s: acc[32 * w:32 * w + 32, s, :], GRP)
                        nc.vector.scalar_tensor_tensor(
                            acc[:], ibig[0:96], float(POW2S * SER[k]), psH[:],
                            op0=MULT, op1=ADD)
                    psL = ps.tile([32, 3, GRP, 32], F32, tag="p3",
                                  name=f"psL{b}_{g}")
                    _persample_round(
                        nc,
                        lambda w, s: psL[:, w, s, :],
                        lambda w, s: etile[32 * w:32 * w + 32, s, :],
                        lambda w, s: acc[32 * w:32 * w + 32, s, :], GRP,
                        tile_col_fn=lambda w: 0)
                    nc.vector.tensor_tensor(lf[:, :, gsl, :], psL[:], ibigl[:],
                                            op=ADD)
                if mhat_dbg is not None:
                    nc.sync.dma_start(dbg["d_mhat"][b], mhat_dbg[:])
                    if m0_dbg is not None:
                        nc.sync.dma_start(dbg["d_m0"][b], m0_dbg[:])
                    if c0_dbg is not None:
                        nc.sync.dma_start(dbg["d_c0"][b], c0_dbg[:])
                    if yz1_dbg is not None:
                        nc.sync.dma_start(dbg["d_yz1"][b], yz1_dbg[:])

                    continue
                if "d_lf" in dbg:
                    nc.sync.dma_start(dbg["d_lf"][b], lf[:])
                if debug_stage <= 3:
                    continue

                # ================= stage C: attention =================
                qrow = bpool.tile([1, 128], F32, tag="qrow", bufs=1)
                krow = bpool.tile([1, 128], F32, tag="krow", bufs=1)
                for kind, row in ((0, qrow), (1, krow)):
                    sqf = bpool.tile([32, P, 32], F32, tag="ytT", bufs=1)
                    nc.vector.tensor_tensor(sqf[:], lf[:, kind], lf[:, kind],
                                            op=MULT)
                    rsf = bpool.tile([32, P], F32, tag="rsf", bufs=1)
                    nc.vector.tensor_reduce(rsf[:], sqf[:],
                                            axis=mybir.AxisListType.X, op=ADD)
                    psq = ps.tile([1, 128], F32, tag="p1c",
                                  name=f"psq{b}_{kind}")
                    nc.tensor.matmul(psq[:], onesc[0:32, 0:1], rsf[:],
                                     start=True, stop=True)
                    nc.scalar.mul(row[:], psq[:], -0.5)
                psE = ps.tile([128, 128], F32, tag="p1c", name=f"psE{b}")
                for j in range(32):
                    nc.tensor.matmul(psE[:], lf[:, 1, :, j], lf[:, 0, :, j],
                                     start=(j == 0), stop=False)
                nc.tensor.matmul(psE[:], onesc[0:1, :], qrow[:],
                                 start=False, stop=False)
                nc.tensor.matmul(psE[:], krow[:], onesc[0:1, :],
                                 start=False, stop=True)
                w1 = bpool.tile([128, 128], F32, tag="w1", bufs=1)
                nc.scalar.activation(w1[:], psE[:],
                                     mybir.ActivationFunctionType.Relu,
                                     scale=-2.0)
                if "d_en" in dbg:
                    nc.sync.dma_start(dbg["d_en"][b], w1[:])
                w2 = bpool.tile([128, 128], F32, tag="w2", bufs=1)
                nc.scalar.activation(w2[:], w1[:],
                                     mybir.ActivationFunctionType.Ln, bias=1.0)
                nc.vector.tensor_scalar_add(w2[:], w2[:], 1.0)
                wr = bpool.tile([128, 128], F32, tag="wr", bufs=1)
                nc.vector.reciprocal(wr[:], w2[:])
                srow = bpool.tile([128, 1], F32, tag="srow", bufs=1)
                ew = bpool.tile([128, 128], F32, tag="ew", bufs=1)
                nc.scalar.activation(ew[:], wr[:],
                                     mybir.ActivationFunctionType.Exp,
                                     accum_out=srow[:])
                rsrow = bpool.tile([128, 1], F32, tag="rsrow", bufs=1)
                nc.vector.reciprocal(rsrow[:], srow[:])
                stile = bpool.tile([128, 128], F32, tag="stile", bufs=1)
                nc.scalar.mul(stile[:], ew[:], rsrow[:])
                if "d_s" in dbg:
                    nc.sync.dma_start(dbg["d_s"][b], stile[:])
                psST = ps.tile([128, 128], F32, tag="p1c", name=f"psST{b}")
                nc.tensor.transpose(psST[:], stile[:], id128[:])
                st_t = bpool.tile([128, 128], F32, tag="st_t", bufs=1)
                nc.scalar.copy(st_t[:], psST[:])
                lvfs = bpool.tile([128, 1024], F32, tag="lvfs", bufs=1)
                nc.sync.dma_start(scrV[:], lf[:, 2])
                nc.sync.dma_start(
                    lvfs[:].rearrange("m (i j) -> m i j", i=32),
                    scrV.rearrange("i m j -> m i j"))
                psML = ps.tile([128, 8, 128], F32, tag="p2a", name=f"psML{b}")
                for c in range(8):
                    nc.tensor.matmul(psML[:, c, :], lvfs[:, 128 * c:128 * (c + 1)],
                                     st_t[:], start=True, stop=True)
                mlT = bpool.tile([128, 8, 128], F32, tag="mlT", bufs=1)
                nc.scalar.copy(mlT[:], psML[:])
                psMT = ps.tile([128, 8, 128], F32, tag="p3", name=f"psMT{b}")
                for c in range(8):
                    nc.tensor.transpose(psMT[:, c, :], mlT[:, c, :], id128[:])
                mlfs = bpool.tile([128, 1024], F32, tag="mlfs", bufs=1)
                nc.scalar.mul(mlfs[:], psMT[:].rearrange("m c e -> m (c e)"),
                              1.0 / (2.0 ** EXP_K))
                if "d_mlfs" in dbg:
                    nc.sync.dma_start(dbg["d_mlfs"][b], mlfs[:])
                if debug_stage <= 4:
                    continue

                # ================= stage D: expm =================
                gt = bpool.tile([128, 32, 32], F32, tag="gt", bufs=1)
                nc.sync.dma_start(scrM[:], mlfs[:])
                for rr in range(4):
                    nc.sync.dma_start(
                        gt[32 * rr:32 * rr + 32, :, :],
                        scrM[rr::4, :].rearrange("g (i j) -> i g j", i=32))
                if "d_gt" in dbg:
                    nc.sync.dma_start(dbg["d_gt"][b], gt[:])
                if debug_stage == 6:
                    continue
                acx = bpool.tile([128, 32, 32], F32, tag="acx", bufs=1)
                nc.vector.tensor_scalar_mul(acx[:], ibgx[:], float(EXP_C[EXP_D]))
                for k in range(EXP_D - 1, -1, -1):
                    psX = ps.tile([128, 32, 32], F32, tag="p2a",
                                  name=f"psXh{b}_{k}")
                    for r in range(4):
                        sl = slice(32 * r, 32 * r + 32)
                        for s in range(32):
                            nc.tensor.matmul(psX[sl, s, :], gt[sl, s, :],
                                             acx[sl, s, :], start=True, stop=True,
                                             tile_position=(32 * r, 32 * r))
                    nc.vector.scalar_tensor_tensor(
                        acx[:], ibgx[:], float(EXP_C[k]), psX[:],
                        op0=MULT, op1=ADD)
                if "d_tay" in dbg:
                    nc.sync.dma_start(dbg["d_tay"][b], acx[:])
                if debug_stage == 7:
                    for rr in range(4):
                        nc.sync.dma_start(
                            out_d[b][rr::4].rearrange("g i j -> i g j"),
                            acx[32 * rr:32 * rr + 32, :, :])
                    continue
                for sq_i in range(EXP_K):
                    psX = ps.tile([128, 32, 32], F32, tag="p2a",
                                  name=f"psXs{b}_{sq_i}")
                    for r in range(4):
                        sl = slice(32 * r, 32 * r + 32)
                        for s in range(32):
                            nc.tensor.matmul(psX[sl, s, :], acx[sl, s, :],
                                             acx[sl, s, :], start=True, stop=True,
                                             tile_position=(32 * r, 32 * r))
                    nc.scalar.copy(acx[:], psX[:])
                for rr in range(4):
                    nc.sync.dma_start(
                        out_d[b][rr::4].rearrange("g i j -> i g j"),
                        acx[32 * rr:32 * rr + 32, :, :])
    nc.compile()
    return nc, dbg


def host_constants(Wq, Wk, Wv):
    wallT = np.concatenate([Wq.T, Wk.T, Wv.T], axis=1).astype(np.float32)
    def w2(W):
        WT = np.ascontiguousarray(W.T.astype(np.float32))
        return np.concatenate([WT[0:32], WT[32:64]], axis=1)
    wall2 = np.concatenate([w2(Wq), w2(Wk), w2(Wv)], axis=0)
    eye = np.eye(32, dtype=np.float32)
    ibig = np.broadcast_to(eye[None, :, None, :],
                           (4, 32, GRP, 32)).reshape(128, GRP, 32).copy()
    ibgx = np.broadcast_to(eye[None, :, None, :],
                           (4, 32, 32, 32)).reshape(128, 32, 32).copy()
    ibigl = (LNC_CONST * np.broadcast_to(
        eye[:, None, None, :], (32, 3, GRP, 32))).reshape(32, 3 * GRP * 32)
    ibigl = np.ascontiguousarray(ibigl, dtype=np.float32)
    id128 = np.eye(128, dtype=np.float32)
    onesc = np.ones((96, 128), dtype=np.float32)
    return {"wallT": wallT, "wall2": wall2, "ibig": ibig, "ibgx": ibgx,
            "ibigl": ibigl, "id128": id128, "onesc": onesc}


_NC_CACHE = {}

def make_in_maps(x, Wq, Wk, Wv):
    consts = host_constants(np.asarray(Wq), np.asarray(Wk), np.asarray(Wv))
    x = np.asarray(x, dtype=np.float32)
    in_maps = []
    for c in range(NCORES):
        m = {"x": np.ascontiguousarray(x[BLOC * c:BLOC * (c + 1)])}
        m.update(consts)
        in_maps.append(m)
    return in_maps


def kernel(x, Wq, Wk, Wv):
    if "full" not in _NC_CACHE:
        _NC_CACHE["full"] = build_nc(99)
    nc, _ = _NC_CACHE["full"]
    in_maps = make_in_maps(x, Wq, Wk, Wv)
    res = run_bass_kernel_spmd(nc, in_maps, list(range(NCORES)))
    out = np.concatenate([res.results[c]["out"] for c in range(NCORES)], axis=0)
    return out.astype(np.float32)



# revision 11
# speedup vs baseline: 1.3880x; 1.3880x over previous
"""Trainium2 Bass kernel for nn_AttentionManifold (B=32, P=128, IN=64, OUT=32).

Data-parallel over batch: each of 8 NeuronCores handles 4 batches.
Per core, per batch:
  A. Q/K/V = W x W^T via two f32r contractions; DVE 32x32 block transpose
     between them. Second contraction deposits per-chunk stacked layout
     [128 part = 4 strips x 32, 16 groups, 32] directly via tile_position.
  B. logm = 4*log((M/c)^(1/4)) + ln(c)*I:
     - two Newton-Schulz sqrt stages (6 + 4 tuned alphas, deferred scalars),
       f32 matmuls on 4 diagonal 32x32 PE tiles, round-robin strip issue.
     - degree-9 shifted-monomial log series evaluated Paterson-Stockmeyer
       style in fp16 (T2, T3 powers + 2 Horner rounds), exit fused with
       the x4 scale and the q0 add (identity-matmul accumulate).
  C. attention: Gram via 32 fp16 accumulating matmuls in [key, query]
     layout, qq/kk via square+reduce and ones-matmul broadcasts, softmax
     along free axis; Frechet rhs matmuls fp16.
  D. expm via scaling-squaring K=2, Taylor degree 8 (Paterson-Stockmeyer:
     A2, A3 powers + 2 Horner rounds) in fp16, squarings in f32.
"""
import math
import numpy as np

import concourse.bacc as bacc
import concourse.mybir as mybir
import concourse.tile as tile
from concourse.bass_utils import run_bass_kernel_spmd

F32 = mybir.dt.float32
F32R = mybir.dt.float32r
F16 = mybir.dt.float16
MULT = mybir.AluOpType.mult
ADD = mybir.AluOpType.add

B, P, IN = 32, 128, 64
NCORES = 8
BLOC = B // NCORES
GRP = 16          # groups per chunk; chunk = 4 strips x GRP samples
NCHUNK = 6        # chunks per batch: 3 kinds x 128 patches / 64

CGLOB = 8.5
LNC = math.log(CGLOB)
A0 = [1.9811481472130752, 1.6297823211553903, 1.4859529112020908,
      1.2452726523328905, 1.0508512265115284, 1.0019755041446587]
A1 = [1.6698279724897085, 1.3264101994958375, 1.0954912251358753,
      1.0070982888317423]
SM, SH = 0.5682121074265004, 0.3743482898558332
MONO = [-0.5652706025589108, 0.6588550507993772, -0.2165382679573131,
        0.09461043190826357, -0.05070406136440695, 0.028508511162318027,
        -0.004575415946334966, 0.00020411832903605154,
        -0.012776160003927822, 0.008539532796427107]
EXP_K = 2
EXP_D = 8
EXP_C = [1.0 / math.factorial(k) for k in range(EXP_D + 1)]


def _stage_scalars():
    out = []
    for st, alphas in enumerate((A0, A1)):
        gammas = []
        p = (1.0 / CGLOB) if st == 0 else 1.0
        q = 1.0
        for a in alphas:
            gammas.append(-(a * a / 3.0) * (p * q))
            p *= 1.5 * a
            q *= 1.5 * a
        out.append((gammas, p))
    return out

STAGE_SCALARS = _stage_scalars()


def _round(nc, out_fn, lhs_fn, rhs_fn, ngrp=GRP, col_fn=None, **mmkw):
    """One per-sample matmul round: strips round-robin inner for LDW overlap."""
    for g in range(ngrp):
        for q in range(4):
            sl = slice(32 * q, 32 * q + 32)
            cq = 32 * q if col_fn is None else col_fn(q)
            nc.tensor.matmul(out_fn(q, g), lhs_fn(sl, g), rhs_fn(sl, g),
                             start=True, stop=True,
                             tile_position=(32 * q, cq), **mmkw)


def build_nc(debug_stage=99):
    nc = bacc.Bacc("TRN2", target_bir_lowering=False, debug=False,
                   num_devices=NCORES)
    x_in = nc.dram_tensor("x", [BLOC, P, IN, IN], F32, kind="ExternalInput").ap()
    wallT_in = nc.dram_tensor("wallT", [IN, 96], F32, kind="ExternalInput").ap()
    wall2_in = nc.dram_tensor("wall2", [96, 64], F32, kind="ExternalInput").ap()
    ibig_in = nc.dram_tensor("ibig", [128, GRP, 32], F32, kind="ExternalInput").ap()
    ibser_in = nc.dram_tensor("ibser", [128, 4, GRP, 32], F16, kind="ExternalInput").ap()
    ident32_in = nc.dram_tensor("ident32", [128, 32], F16, kind="ExternalInput").ap()
    ibexp_in = nc.dram_tensor("ibexp", [128, 3, 32, 32], F16, kind="ExternalInput").ap()
    id128_in = nc.dram_tensor("id128", [128, 128], F32, kind="ExternalInput").ap()
    ones_in = nc.dram_tensor("onesc", [96, 128], F32, kind="ExternalInput").ap()
    out_d = nc.dram_tensor("out", [BLOC, P, 32, 32], F32, kind="ExternalOutput").ap()

    dbg = {}
    def dbg_out(name, shape):
        dbg[name] = nc.dram_tensor(name, shape, F32, kind="ExternalOutput").ap()
    if debug_stage == 1:
        dbg_out("d_mats", [BLOC, NCHUNK, 128, GRP, 32])
    if debug_stage == 2:
        dbg_out("d_m1", [BLOC, NCHUNK, 128, GRP, 32])
    if debug_stage == 3:
        dbg_out("d_lf", [BLOC, 32, 3, P, 32])
    if debug_stage == 4:
        dbg_out("d_s", [BLOC, 128, 128])
        dbg_out("d_mlfs", [BLOC, 128, 1024])
    if debug_stage == 5:
        dbg_out("d_gt", [BLOC, 128, 32, 32])

    c9 = MONO[9]

    with tile.TileContext(nc) as tc:
        with (
            tc.tile_pool(name="const", bufs=1) as cpool,
            tc.tile_pool(name="perb", bufs=1) as bpool,
            tc.tile_pool(name="chk", bufs=2) as kpool,
            tc.tile_pool(name="ps", bufs=1, space="PSUM") as ps,
            tc.tile_pool(name="dscr", bufs=1, space="DRAM") as dpool,
        ):
            scrV_t = dpool.tile([32, P, 32], F16, name="scrV")
            scrM_t = dpool.tile([P, 1024], F16, name="scrM")
            scrV = scrV_t[:]
            scrM = scrM_t[:]
            wallT = cpool.tile([IN, 96], F32)
            nc.sync.dma_start(wallT[:], wallT_in[:])
            wallTr = cpool.tile([IN, 96], F32R)
            nc.vector.tensor_copy(wallTr[:], wallT[:])
            wall2 = cpool.tile([96, 2, 32], F32)
            nc.sync.dma_start(wall2[:], wall2_in.rearrange("p (h j) -> p h j", h=2))
            ibig = cpool.tile([128, GRP, 32], F32)
            nc.sync.dma_start(ibig[:], ibig_in[:])
            ibser = cpool.tile([128, 4, GRP, 32], F16)
            nc.sync.dma_start(ibser[:], ibser_in[:])
            ident32 = cpool.tile([128, 32], F16)
            nc.sync.dma_start(ident32[:], ident32_in[:])
            ibexp = cpool.tile([128, 3, 32, 32], F16)
            nc.sync.dma_start(ibexp[:], ibexp_in[:])
            id128 = cpool.tile([128, 128], F32)
            nc.sync.dma_start(id128[:], id128_in[:])
            onesc = cpool.tile([96, 128], F32)
            nc.sync.dma_start(onesc[:], ones_in[:])

            for b in range(BLOC):
                # ================= stage A: first contraction =================
                xt = bpool.tile([IN, P, IN], F32, tag="xt", bufs=1)
                nc.sync.dma_start(xt[:], x_in[b].rearrange("p i j -> i p j"))
                ytT = bpool.tile([96, P, 2, 32], F32, tag="ytT", bufs=1)
                for t in range(16):
                    xs = xt[:].rearrange("i p j -> i (p j)")[:, 512 * t:512 * (t + 1)]
                    xr = kpool.tile([IN, 512], F32R, tag="xr")
                    nc.vector.tensor_copy(xr[:], xs)
                    psY = ps.tile([96, 512], F32, tag="pA", bufs=2, name=f"psY{b}_{t}")
                    nc.tensor.matmul(psY[:], wallTr[:], xr[:], start=True, stop=True)
                    nc.vector.transpose(
                        ytT[:].rearrange("p m h j -> p (m h j)")[:, 512 * t:512 * (t + 1)],
                        psY[:])

                lf = bpool.tile([32, 3, P, 32], F16, tag="lf", bufs=1)

                for c in range(NCHUNK):
                    w = c // 2
                    poff = 64 * (c % 2)
                    wsl = slice(32 * w, 32 * w + 32)
                    # ======== stage A2: entry (psB = W (Sx W^T) per chunk) ====
                    psB = ps.tile([128, GRP, 32], F32, tag="pA", bufs=2, name=f"psB{b}_{c}")
                    for q in range(4):
                        for h in range(2):
                            nc.tensor.matmul(
                                psB[32 * q:32 * q + 32, :, :],
                                wall2[wsl, h, :],
                                ytT[wsl, poff + q:poff + 64:4, h, :],
                                start=(h == 0), stop=(h == 1),
                                tile_position=(32 * w, 32 * q))
                    if "d_mats" in dbg:
                        st_dbg = kpool.tile([128, GRP, 32], F32, tag="stdbg")
                        nc.vector.tensor_copy(st_dbg[:], psB[:])
                        nc.sync.dma_start(dbg["d_mats"][b, c], st_dbg[:])
                    if debug_stage <= 1:
                        continue

                    # ================= stage B: logm =================
                    m0 = kpool.tile([128, GRP, 32], F32, tag="m0")
                    nc.scalar.copy(m0[:], psB[:])
                    yz = kpool.tile([128, GRP, 64], F32, tag="yz")
                    ctile = kpool.tile([128, GRP, 32], F32, tag="ctile")
                    for st in range(2):
                        gammas, pn = STAGE_SCALARS[st]
                        if st == 0:
                            src = psB
                            sc = 1.0
                        else:
                            src = psYZ
                            sc = STAGE_SCALARS[0][1]   # pn0
                            nc.scalar.mul(m0[:], psYZ[:, :, 0:32], float(sc))
                        nc.vector.scalar_tensor_tensor(
                            ctile[:], src[:, :, 0:32] if st else src[:],
                            float(gammas[0] * sc), ibig[:], op0=MULT, op1=ADD)
                        psYZ = ps.tile([128, GRP, 64], F32, tag="pB", bufs=2,
                                       name=f"psYZ0_{b}_{c}_{st}")
                        _round(nc, lambda q, g: psYZ[32 * q:32 * q + 32, g, 0:32],
                               lambda sl, g: ctile[sl, g, :],
                               lambda sl, g: m0[sl, g, :])
                        nc.scalar.copy(yz[:, :, 0:32], psYZ[:, :, 0:32])
                        nc.vector.tensor_copy(yz[:, :, 32:64], ctile[:])
                        for k in range(1, len(gammas)):
                            psT = ps.tile([128, GRP, 32], F32, tag="pA", bufs=2,
                                          name=f"psT{b}_{c}_{st}_{k}")
                            _round(nc, lambda q, g: psT[32 * q:32 * q + 32, g, :],
                                   lambda sl, g: yz[sl, g, 32:64],
                                   lambda sl, g: yz[sl, g, 0:32])
                            nc.vector.scalar_tensor_tensor(
                                ctile[:], psT[:], float(gammas[k]), ibig[:],
                                op0=MULT, op1=ADD)
                            psYZ = ps.tile([128, GRP, 64], F32, tag="pB", bufs=2,
                                           name=f"psYZ{b}_{c}_{st}_{k}")
                            _round(nc, lambda q, g: psYZ[32 * q:32 * q + 32, g, :],
                                   lambda sl, g: ctile[sl, g, :],
                                   lambda sl, g: yz[sl, g, :])
                            if k < len(gammas) - 1:
                                if k % 2:
                                    nc.scalar.copy(yz[:], psYZ[:])
                                else:
                                    nc.vector.tensor_copy(yz[:], psYZ[:])
                    if "d_m1" in dbg:
                        m1_dbg = kpool.tile([128, GRP, 32], F32, tag="stdbg")
                        nc.scalar.mul(m1_dbg[:], psYZ[:, :, 0:32],
                                      float(STAGE_SCALARS[1][1]))
                        nc.sync.dma_start(dbg["d_m1"][b, c], m1_dbg[:])
                    if debug_stage <= 2:
                        continue

                    # ---- series: T = (pn1*y - SM)/SH, PS deg-9, fp16 ----
                    pn1 = STAGE_SCALARS[1][1]
                    tser = kpool.tile([128, GRP, 32], F16, tag="tser")
                    nc.vector.scalar_tensor_tensor(
                        tser[:], psYZ[:, :, 0:32], float(pn1 / SH),
                        ibser[:, 3], op0=MULT, op1=ADD)
                    psP = ps.tile([128, GRP, 32], F32, tag="pA", bufs=2,
                                  name=f"psT2_{b}_{c}")
                    _round(nc, lambda q, g: psP[32 * q:32 * q + 32, g, :],
                           lambda sl, g: tser[sl, g, :],
                           lambda sl, g: tser[sl, g, :])
                    t2 = kpool.tile([128, GRP, 32], F16, tag="t2")
                    nc.scalar.copy(t2[:], psP[:])
                    psP = ps.tile([128, GRP, 32], F32, tag="pA", bufs=2,
                                  name=f"psT3_{b}_{c}")
                    _round(nc, lambda q, g: psP[32 * q:32 * q + 32, g, :],
                           lambda sl, g: tser[sl, g, :],
                           lambda sl, g: t2[sl, g, :])
                    t3 = kpool.tile([128, GRP, 32], F16, tag="t3")
                    nc.scalar.copy(t3[:], psP[:])
                    # q_i builds (fp16 DVE)
                    qt = [kpool.tile([128, GRP, 32], F16, tag=f"q{i}",
                                     name=f"qt{i}_{b}_{c}")
                          for i in range(3)]
                    for i in range(3):
                        nc.vector.scalar_tensor_tensor(
                            qt[i][:], tser[:], float(MONO[3 * i + 1]),
                            ibser[:, i], op0=MULT, op1=ADD)
                        nc.vector.scalar_tensor_tensor(
                            qt[i][:], t2[:], float(MONO[3 * i + 2]), qt[i][:],
                            op0=MULT, op1=ADD)
                    h1 = kpool.tile([128, GRP, 32], F16, tag="h1")
                    nc.vector.scalar_tensor_tensor(
                        h1[:], t3[:], float(c9), qt[2][:], op0=MULT, op1=ADD)
                    psP = ps.tile([128, GRP, 32], F32, tag="pA", bufs=2,
                                  name=f"psH_{b}_{c}")
                    _round(nc, lambda q, g: psP[32 * q:32 * q + 32, g, :],
                           lambda sl, g: h1[sl, g, :],
                           lambda sl, g: t3[sl, g, :])
                    acc1 = kpool.tile([128, GRP, 32], F16, tag="acc1")
                    nc.vector.tensor_tensor(acc1[:], psP[:], qt[1][:], op=ADD)
                    # final Horner round + q0 add, output to strips 0-31
                    for half in range(2):
                        psL = ps.tile([32, 2, GRP, 32], F32, tag="pC",
                                      name=f"psL{b}_{c}_{half}")
                        # q0 first (start=True sets has_written across the
                        # region), then Horner matmuls accumulate onto it.
                        for ql in range(2):
                            q = 2 * half + ql
                            sl = slice(32 * q, 32 * q + 32)
                            nc.tensor.matmul(
                                psL[:, ql, :, :], ident32[sl, :],
                                qt[0][sl, :, :], start=True, stop=False,
                                tile_position=(32 * q, 0),
                                skip_group_check=True)
                        for g in range(GRP):
                            for ql in range(2):
                                q = 2 * half + ql
                                sl = slice(32 * q, 32 * q + 32)
                                nc.tensor.matmul(
                                    psL[:, ql, g, :], acc1[sl, g, :],
                                    t3[sl, g, :], start=False,
                                    stop=(g == GRP - 1),
                                    tile_position=(32 * q, 0),
                                    skip_group_check=True)
                        dst = lf[:, w, poff:poff + 64, :].rearrange(
                            "m (g f) j -> m f g j", f=4)[:, 2 * half:2 * half + 2]
                        nc.vector.tensor_scalar_mul(dst, psL[:], 4.0)

                if debug_stage <= 2:
                    continue
                if "d_lf" in dbg:
                    lff = bpool.tile([32, 3, P, 32], F32, tag="lff", bufs=1)
                    nc.vector.tensor_copy(lff[:], lf[:])
                    nc.sync.dma_start(dbg["d_lf"][b], lff[:])
                if debug_stage <= 3:
                    continue

                # ================= stage C: attention =================
                qrow = bpool.tile([1, 128], F32, tag="qrow", bufs=1)
                krow = bpool.tile([1, 128], F32, tag="krow", bufs=1)
                for kind, row in ((0, qrow), (1, krow)):
                    sqf = bpool.tile([32, P, 32], F16, tag="xt", bufs=1)
                    nc.scalar.activation(sqf[:], lf[:, kind],
                                         mybir.ActivationFunctionType.Square)
                    rsf = bpool.tile([32, P], F32, tag="rsf", bufs=1)
                    nc.vector.tensor_reduce(rsf[:], sqf[:],
                                            axis=mybir.AxisListType.X, op=ADD)
                    psq = ps.tile([1, 128], F32, tag="pA", bufs=2,
                                  name=f"psq{b}_{kind}")
                    nc.tensor.matmul(psq[:], onesc[0:32, 0:1], rsf[:],
                                     start=True, stop=True)
                    nc.scalar.mul(row[:], psq[:], -0.5)
                psE = ps.tile([128, 128], F32, tag="pB", bufs=2, name=f"psE{b}")
                for j in range(32):
                    nc.tensor.matmul(psE[:], lf[:, 1, :, j], lf[:, 0, :, j],
                                     start=(j == 0), stop=False)
                nc.tensor.matmul(psE[:], onesc[0:1, :], qrow[:],
                                 start=False, stop=False)
                nc.tensor.matmul(psE[:], krow[:], onesc[0:1, :],
                                 start=False, stop=True)
                w1 = bpool.tile([128, 128], F32, tag="w1", bufs=1)
                nc.scalar.activation(w1[:], psE[:],
                                     mybir.ActivationFunctionType.Relu,
                                     scale=-2.0)
                w2 = bpool.tile([128, 128], F32, tag="w2", bufs=1)
                nc.scalar.activation(w2[:], w1[:],
                                     mybir.ActivationFunctionType.Ln, bias=1.0)
                nc.vector.tensor_scalar_add(w2[:], w2[:], 1.0)
                wr = bpool.tile([128, 128], F32, tag="wr", bufs=1)
                nc.vector.reciprocal(wr[:], w2[:])
                srow = bpool.tile([128, 1], F32, tag="srow", bufs=1)
                ew = bpool.tile([128, 128], F32, tag="ew", bufs=1)
                nc.scalar.activation(ew[:], wr[:],
                                     mybir.ActivationFunctionType.Exp,
                                     accum_out=srow[:])
                rsrow = bpool.tile([128, 1], F32, tag="rsrow", bufs=1)
                nc.vector.reciprocal(rsrow[:], srow[:])
                stile = bpool.tile([128, 128], F32, tag="stile", bufs=1)
                nc.scalar.mul(stile[:], ew[:], rsrow[:])
                if "d_s" in dbg:
                    nc.sync.dma_start(dbg["d_s"][b], stile[:])
                psST = ps.tile([128, 128], F32, tag="pA", bufs=2, name=f"psST{b}")
                nc.tensor.transpose(psST[:], stile[:], id128[:])
                st_t = bpool.tile([128, 128], F16, tag="st_t", bufs=1)
                nc.scalar.copy(st_t[:], psST[:])
                lvfs = bpool.tile([128, 1024], F16, tag="lvfs", bufs=1)
                nc.sync.dma_start(scrV[:], lf[:, 2])
                nc.sync.dma_start(
                    lvfs[:].rearrange("m (i j) -> m i j", i=32),
                    scrV.rearrange("i m j -> m i j"))
                psML = ps.tile([128, 8, 128], F32, tag="pB", bufs=2, name=f"psML{b}")
                for cc in range(8):
                    nc.tensor.matmul(psML[:, cc, :], lvfs[:, 128 * cc:128 * (cc + 1)],
                                     st_t[:], start=True, stop=True)
                mlT = bpool.tile([128, 8, 128], F32, tag="mlT", bufs=1)
                nc.scalar.copy(mlT[:], psML[:])
                psMT = ps.tile([128, 8, 128], F32, tag="pB", bufs=2, name=f"psMT{b}")
                for cc in range(8):
                    nc.tensor.transpose(psMT[:, cc, :], mlT[:, cc, :], id128[:])
                mlfs = bpool.tile([128, 1024], F16, tag="mlfs", bufs=1)
                nc.scalar.mul(mlfs[:], psMT[:].rearrange("m c e -> m (c e)"),
                              1.0 / (2.0 ** EXP_K))
                if "d_mlfs" in dbg:
                    mlfs_f = bpool.tile([128, 1024], F32, tag="mlfsf", bufs=1)
                    nc.vector.tensor_copy(mlfs_f[:], mlfs[:])
                    nc.sync.dma_start(dbg["d_mlfs"][b], mlfs_f[:])
                if debug_stage <= 4:
                    continue

                # ================= stage D: expm =================
                gt = bpool.tile([128, 32, 32], F16, tag="gt", bufs=1)
                nc.sync.dma_start(scrM[:], mlfs[:])
                for rr in range(4):
                    nc.sync.dma_start(
                        gt[32 * rr:32 * rr + 32, :, :],
                        scrM[rr::4, :].rearrange("g (i j) -> i g j", i=32))
                if "d_gt" in dbg:
                    gt_f = bpool.tile([128, 32, 32], F32, tag="gtf", bufs=1)
                    nc.vector.tensor_copy(gt_f[:], gt[:])
                    nc.sync.dma_start(dbg["d_gt"][b], gt_f[:])
                if debug_stage == 5:
                    continue
                psX = ps.tile([128, 32, 32], F32, tag="pB", bufs=2, name=f"psA2{b}")
                _round(nc, lambda q, g: psX[32 * q:32 * q + 32, g, :],
                       lambda sl, g: gt[sl, g, :],
                       lambda sl, g: gt[sl, g, :], ngrp=32)
                a2 = bpool.tile([128, 32, 32], F16, tag="a2", bufs=1)
                nc.scalar.copy(a2[:], psX[:])
                psX = ps.tile([128, 32, 32], F32, tag="pB", bufs=2, name=f"psA3{b}")
                _round(nc, lambda q, g: psX[32 * q:32 * q + 32, g, :],
                       lambda sl, g: gt[sl, g, :],
                       lambda sl, g: a2[sl, g, :], ngrp=32)
                a3 = bpool.tile([128, 32, 32], F16, tag="a3", bufs=1)
                nc.scalar.copy(a3[:], psX[:])
                # q_i = ibexp[i] + c_{3i+1} A + c_{3i+2} A2  (Taylor, PS)
                qe = [bpool.tile([128, 32, 32], F16, tag=f"qe{i}", bufs=1,
                                 name=f"qe{i}_{b}")
                      for i in range(3)]
                for i in range(3):
                    nc.vector.scalar_tensor_tensor(
                        qe[i][:], gt[:], float(EXP_C[3 * i + 1]), ibexp[:, i],
                        op0=MULT, op1=ADD)
                    nc.vector.scalar_tensor_tensor(
                        qe[i][:], a2[:], float(EXP_C[3 * i + 2]), qe[i][:],
                        op0=MULT, op1=ADD)
                psX = ps.tile([128, 32, 32], F32, tag="pB", bufs=2, name=f"psH1{b}")
                _round(nc, lambda q, g: psX[32 * q:32 * q + 32, g, :],
                       lambda sl, g: qe[2][sl, g, :],
                       lambda sl, g: a3[sl, g, :], ngrp=32)
                eacc = bpool.tile([128, 32, 32], F16, tag="eacc", bufs=1)
                nc.vector.tensor_tensor(eacc[:], psX[:], qe[1][:], op=ADD)
                psX = ps.tile([128, 32, 32], F32, tag="pB", bufs=2, name=f"psH2{b}")
                _round(nc, lambda q, g: psX[32 * q:32 * q + 32, g, :],
                       lambda sl, g: eacc[sl, g, :],
                       lambda sl, g: a3[sl, g, :], ngrp=32)
                sq0 = bpool.tile([128, 32, 32], F32, tag="sq0", bufs=1)
                nc.vector.tensor_tensor(sq0[:], psX[:], qe[0][:], op=ADD)
                # squarings in f32
                for sq_i in range(EXP_K):
                    psX = ps.tile([128, 32, 32], F32, tag="pB", bufs=2,
                                  name=f"psS{b}_{sq_i}")
                    _round(nc, lambda q, g: psX[32 * q:32 * q + 32, g, :],
                           lambda sl, g: sq0[sl, g, :],
                           lambda sl, g: sq0[sl, g, :], ngrp=32)
                    if sq_i < EXP_K - 1:
                        nc.scalar.copy(sq0[:], psX[:])
                    else:
                        nc.vector.tensor_copy(sq0[:], psX[:])
                for rr in range(4):
                    nc.sync.dma_start(
                        out_d[b][rr::4].rearrange("g i j -> i g j"),
                        sq0[32 * rr:32 * rr + 32, :, :])
    nc.compile()
    return nc, dbg


def host_constants(Wq, Wk, Wv):
    wallT = np.concatenate([Wq.T, Wk.T, Wv.T], axis=1).astype(np.float32)
    def w2(W):
        WT = np.ascontiguousarray(W.T.astype(np.float32))
        return np.concatenate([WT[0:32], WT[32:64]], axis=1)
    wall2 = np.concatenate([w2(Wq), w2(Wk), w2(Wv)], axis=0)
    eye = np.eye(32, dtype=np.float32)
    ibig = np.broadcast_to(eye[None, :, None, :],
                           (4, 32, GRP, 32)).reshape(128, GRP, 32).copy()
    # ibser[:, i] = s_i * I for i in 0..2 (series q_i consts), 3 = -SM/SH
    scal = [MONO[0] + LNC / 4.0, MONO[3], MONO[6], -SM / SH]
    ibser = np.stack([s * ibig for s in scal], axis=1).astype(np.float16)
    ident32 = np.broadcast_to(eye[None], (4, 32, 32)).reshape(128, 32).astype(np.float16)
    ibgx = np.broadcast_to(eye[None, :, None, :],
                           (4, 32, 32, 32)).reshape(128, 32, 32).copy()
    escal = [EXP_C[0], EXP_C[3], EXP_C[6]]
    ibexp = np.stack([s * ibgx for s in escal], axis=1).astype(np.float16)
    id128 = np.eye(128, dtype=np.float32)
    onesc = np.ones((96, 128), dtype=np.float32)
    return {"wallT": wallT, "wall2": wall2, "ibig": ibig, "ibser": ibser,
            "ident32": ident32, "ibexp": ibexp, "id128": id128,
            "onesc": onesc}


_NC_CACHE = {}

def make_in_maps(x, Wq, Wk, Wv):
    consts = host_constants(np.asarray(Wq), np.asarray(Wk), np.asarray(Wv))
    x = np.asarray(x, dtype=np.float32)
    in_maps = []
    for c in range(NCORES):
        m = {"x": np.ascontiguousarray(x[BLOC * c:BLOC * (c + 1)])}
        m.update(consts)
        in_maps.append(m)
    return in_maps


def kernel(x, Wq, Wk, Wv):
    if "full" not in _NC_CACHE:
        _NC_CACHE["full"] = build_nc(99)
    nc, _ = _NC_CACHE["full"]
    in_maps = make_in_maps(x, Wq, Wk, Wv)
    res = run_bass_kernel_spmd(nc, in_maps, list(range(NCORES)))
    out = np.concatenate([res.results[c]["out"] for c in range(NCORES)], axis=0)
    return out.astype(np.float32)


# revision 15
# speedup vs baseline: 4.4825x; 3.2294x over previous
"""Trainium2 Bass kernel for nn_AttentionManifold (B=32, P=128, IN=64, OUT=32).

Data-parallel over batch: each of 8 NeuronCores handles 4 batches.
Per core, per batch:
  A. Q/K/V = W x W^T via two f32r contractions; DVE 32x32 block transpose
     between them. Second contraction deposits per-chunk stacked layout
     [128 part = 4 strips x 32, 16 groups, 32] directly via tile_position.
  B. logm = 4*log((M/c)^(1/4)) + ln(c)*I:
     - two Newton-Schulz sqrt stages (6 + 4 tuned alphas, deferred scalars),
       f32 matmuls on 4 diagonal 32x32 PE tiles, round-robin strip issue.
     - degree-9 shifted-monomial log series evaluated Paterson-Stockmeyer
       style in fp16 (T2, T3 powers + 2 Horner rounds), exit fused with
       the x4 scale and the q0 add (identity-matmul accumulate).
  C. attention: Gram via 32 fp16 accumulating matmuls in [key, query]
     layout, qq/kk via square+reduce and ones-matmul broadcasts, softmax
     along free axis; Frechet rhs matmuls fp16.
  D. expm via scaling-squaring K=2, Taylor degree 8 (Paterson-Stockmeyer:
     A2, A3 powers + 2 Horner rounds) in fp16, squarings in f32.
"""
import math
import numpy as np

import concourse.bacc as bacc
import concourse.mybir as mybir
import concourse.tile as tile
from concourse.bass_utils import run_bass_kernel_spmd

F32 = mybir.dt.float32
F32R = mybir.dt.float32r
F16 = mybir.dt.float16
MULT = mybir.AluOpType.mult
ADD = mybir.AluOpType.add

B, P, IN = 32, 128, 64
NCORES = 8
BLOC = B // NCORES
GRP = 16          # groups per chunk; chunk = 4 strips x GRP samples
NCHUNK = 6        # chunks per batch: 3 kinds x 128 patches / 64

CGLOB = 8.5
LNC = math.log(CGLOB)
A0 = [1.9811481472130752, 1.6297823211553903, 1.4859529112020908,
      1.2452726523328905, 1.0508512265115284, 1.0019755041446587]
A1 = [1.6698279724897085, 1.3264101994958375, 1.0954912251358753,
      1.0070982888317423]
SM, SH = 0.5682121074265004, 0.3743482898558332
MONO = [-0.5652706025589108, 0.6588550507993772, -0.2165382679573131,
        0.09461043190826357, -0.05070406136440695, 0.028508511162318027,
        -0.004575415946334966, 0.00020411832903605154,
        -0.012776160003927822, 0.008539532796427107]
EXP_K = 2
EXP_D = 8
EXP_C = [1.0 / math.factorial(k) for k in range(EXP_D + 1)]


def _stage_scalars():
    out = []
    for st, alphas in enumerate((A0, A1)):
        gammas = []
        p = (1.0 / CGLOB) if st == 0 else 1.0
        q = 1.0
        for a in alphas:
            gammas.append(-(a * a / 3.0) * (p * q))
            p *= 1.5 * a
            q *= 1.5 * a
        out.append((gammas, p))
    return out

STAGE_SCALARS = _stage_scalars()


def _round(nc, out_fn, lhs_fn, rhs_fn, ngrp=GRP, col_fn=None, **mmkw):
    """One per-sample matmul round: strips round-robin inner for LDW overlap."""
    for g in range(ngrp):
        for q in range(4):
            sl = slice(32 * q, 32 * q + 32)
            cq = 32 * q if col_fn is None else col_fn(q)
            nc.tensor.matmul(out_fn(q, g), lhs_fn(sl, g), rhs_fn(sl, g),
                             start=True, stop=True,
                             tile_position=(32 * q, cq), **mmkw)


def build_nc(debug_stage=99, with_dbg=True):
    nc = bacc.Bacc("TRN2", target_bir_lowering=False, debug=False,
                   num_devices=NCORES)
    x_in = nc.dram_tensor("x", [BLOC, P, IN, IN], F32, kind="ExternalInput").ap()
    wallT_in = nc.dram_tensor("wallT", [IN, 96], F32, kind="ExternalInput").ap()
    wall2_in = nc.dram_tensor("wall2", [96, 64], F32, kind="ExternalInput").ap()
    ibig_in = nc.dram_tensor("ibig", [128, GRP, 32], F32, kind="ExternalInput").ap()
    ibser_in = nc.dram_tensor("ibser", [128, 4, GRP, 32], F16, kind="ExternalInput").ap()
    ident32_in = nc.dram_tensor("ident32", [128, 32], F16, kind="ExternalInput").ap()
    ibexp_in = nc.dram_tensor("ibexp", [128, 3, 32, 32], F16, kind="ExternalInput").ap()
    id128_in = nc.dram_tensor("id128", [128, 128], F32, kind="ExternalInput").ap()
    ones_in = nc.dram_tensor("onesc", [96, 128], F32, kind="ExternalInput").ap()
    out_d = nc.dram_tensor("out", [BLOC, P, 32, 32], F32, kind="ExternalOutput").ap()

    dbg = {}
    def dbg_out(name, shape):
        if not with_dbg:
            return
        dbg[name] = nc.dram_tensor(name, shape, F32, kind="ExternalOutput").ap()
    if debug_stage == 1:
        dbg_out("d_mats", [BLOC, NCHUNK, 128, GRP, 32])
    if debug_stage == 2:
        dbg_out("d_m1", [BLOC, NCHUNK, 128, GRP, 32])
    if debug_stage == 3:
        dbg_out("d_lf", [BLOC, 32, 3, P, 32])
    if debug_stage == 4:
        dbg_out("d_s", [BLOC, 128, 128])
        dbg_out("d_mlfs", [BLOC, 128, 1024])
    if debug_stage == 5:
        dbg_out("d_gt", [BLOC, 128, 32, 32])

    c9 = MONO[9]

    with tile.TileContext(nc) as tc:
        with (
            tc.tile_pool(name="const", bufs=1) as cpool,
            tc.tile_pool(name="perb", bufs=1) as bpool,
            tc.tile_pool(name="chk", bufs=2) as kpool,
            tc.tile_pool(name="ps", bufs=1, space="PSUM") as ps,
            tc.tile_pool(name="dscr", bufs=1, space="DRAM") as dpool,
        ):
            scrV_t = dpool.tile([32, P, 32], F16, name="scrV")
            scrM_t = dpool.tile([P, 1024], F16, name="scrM")
            scrV = scrV_t[:]
            scrM = scrM_t[:]
            wallT = cpool.tile([IN, 96], F32)
            nc.sync.dma_start(wallT[:], wallT_in[:])
            wallTr = cpool.tile([IN, 96], F32R)
            nc.vector.tensor_copy(wallTr[:], wallT[:])
            wall2 = cpool.tile([96, 2, 32], F32)
            nc.sync.dma_start(wall2[:], wall2_in.rearrange("p (h j) -> p h j", h=2))
            ibig = cpool.tile([128, GRP, 32], F32)
            nc.sync.dma_start(ibig[:], ibig_in[:])
            ibser = cpool.tile([128, 4, GRP, 32], F16)
            nc.sync.dma_start(ibser[:], ibser_in[:])
            ident32 = cpool.tile([128, 32], F16)
            nc.sync.dma_start(ident32[:], ident32_in[:])
            ibexp = cpool.tile([128, 3, 32, 32], F16)
            nc.sync.dma_start(ibexp[:], ibexp_in[:])
            id128 = cpool.tile([128, 128], F32)
            nc.sync.dma_start(id128[:], id128_in[:])
            onesc = cpool.tile([96, 128], F32)
            nc.sync.dma_start(onesc[:], ones_in[:])

            for b in range(BLOC):
                # ================= stage A: first contraction =================
                xt = bpool.tile([IN, P, IN], F32, tag="xt", bufs=1)
                nc.sync.dma_start(xt[:], x_in[b].rearrange("p i j -> i p j"))
                ytT = bpool.tile([96, P, 2, 32], F32, tag="ytT", bufs=1)
                for t in range(16):
                    xs = xt[:].rearrange("i p j -> i (p j)")[:, 512 * t:512 * (t + 1)]
                    xr = kpool.tile([IN, 512], F32R, tag="xr")
                    nc.vector.tensor_copy(xr[:], xs)
                    psY = ps.tile([96, 512], F32, tag="pA", bufs=3, name=f"psY{b}_{t}")
                    nc.tensor.matmul(psY[:], wallTr[:], xr[:], start=True, stop=True)
                    nc.vector.transpose(
                        ytT[:].rearrange("p m h j -> p (m h j)")[:, 512 * t:512 * (t + 1)],
                        psY[:])

                lf = bpool.tile([32, 3, P, 32], F16, tag="lf", bufs=1)

                def chunk_gen(c):
                    w = c // 2
                    poff = 64 * (c % 2)
                    wsl = slice(32 * w, 32 * w + 32)
                    psB = ps.tile([128, GRP, 32], F32, tag="pA", bufs=3,
                                  name=f"psB{b}_{c}")
                    for q in range(4):
                        for h in range(2):
                            nc.tensor.matmul(
                                psB[32 * q:32 * q + 32, :, :],
                                wall2[wsl, h, :],
                                ytT[wsl, poff + q:poff + 64:4, h, :],
                                start=(h == 0), stop=(h == 1),
                                tile_position=(32 * w, 32 * q))
                    if "d_mats" in dbg:
                        st_dbg = kpool.tile([128, GRP, 32], F32, tag="stdbg",
                                            name=f"sd{b}_{c}")
                        nc.vector.tensor_copy(st_dbg[:], psB[:])
                        nc.sync.dma_start(dbg["d_mats"][b, c], st_dbg[:])
                    if debug_stage <= 1:
                        return
                    yield
                    # ================= stage B: logm =================
                    m0 = kpool.tile([128, GRP, 32], F32, tag="m0",
                                    name=f"m0_{b}_{c}")
                    nc.scalar.copy(m0[:], psB[:])
                    yz = kpool.tile([128, GRP, 64], F32, tag="yz",
                                    name=f"yz_{b}_{c}")
                    ctile = kpool.tile([128, GRP, 32], F32, tag="ctile",
                                       name=f"ct_{b}_{c}")
                    psYZ = None
                    for st in range(2):
                        gammas, pn = STAGE_SCALARS[st]
                        if st == 0:
                            nc.vector.scalar_tensor_tensor(
                                ctile[:], psB[:], float(gammas[0]), ibig[:],
                                op0=MULT, op1=ADD)
                        else:
                            sc = STAGE_SCALARS[0][1]   # pn0
                            nc.scalar.mul(m0[:], psYZ[:, :, 0:32], float(sc))
                            nc.vector.scalar_tensor_tensor(
                                ctile[:], psYZ[:, :, 0:32],
                                float(gammas[0] * sc), ibig[:],
                                op0=MULT, op1=ADD)
                        yield
                        psYZ = ps.tile([128, GRP, 64], F32, tag="pB", bufs=2,
                                       name=f"psYZ0_{b}_{c}_{st}")
                        _round(nc, lambda q, g: psYZ[32 * q:32 * q + 32, g, 0:32],
                               lambda sl, g: ctile[sl, g, :],
                               lambda sl, g: m0[sl, g, :])
                        nc.scalar.copy(yz[:, :, 0:32], psYZ[:, :, 0:32])
                        nc.scalar.copy(yz[:, :, 32:64], ctile[:])
                        yield
                        for k in range(1, len(gammas)):
                            psT = ps.tile([128, GRP, 32], F32, tag="pA", bufs=3,
                                          name=f"psT{b}_{c}_{st}_{k}")
                            _round(nc, lambda q, g: psT[32 * q:32 * q + 32, g, :],
                                   lambda sl, g: yz[sl, g, 32:64],
                                   lambda sl, g: yz[sl, g, 0:32])
                            nc.vector.scalar_tensor_tensor(
                                ctile[:], psT[:], float(gammas[k]), ibig[:],
                                op0=MULT, op1=ADD)
                            yield
                            psYZ = ps.tile([128, GRP, 64], F32, tag="pB", bufs=2,
                                           name=f"psYZ{b}_{c}_{st}_{k}")
                            _round(nc, lambda q, g: psYZ[32 * q:32 * q + 32, g, :],
                                   lambda sl, g: ctile[sl, g, :],
                                   lambda sl, g: yz[sl, g, :])
                            if k < len(gammas) - 1:
                                nc.scalar.copy(yz[:], psYZ[:])
                            yield
                    if "d_m1" in dbg:
                        m1_dbg = kpool.tile([128, GRP, 32], F32, tag="stdbg",
                                            name=f"m1d{b}_{c}")
                        nc.scalar.mul(m1_dbg[:], psYZ[:, :, 0:32],
                                      float(STAGE_SCALARS[1][1]))
                        nc.sync.dma_start(dbg["d_m1"][b, c], m1_dbg[:])
                    if debug_stage <= 2:
                        return

                    # ---- series: T = (pn1*y - SM)/SH, PS deg-9, fp16 ----
                    pn1 = STAGE_SCALARS[1][1]
                    tser = kpool.tile([128, GRP, 32], F16, tag="tser",
                                      name=f"ts_{b}_{c}")
                    nc.vector.scalar_tensor_tensor(
                        tser[:], psYZ[:, :, 0:32], float(pn1 / SH),
                        ibser[:, 3], op0=MULT, op1=ADD)
                    yield
                    psP = ps.tile([128, GRP, 32], F32, tag="pA", bufs=3,
                                  name=f"psT2_{b}_{c}")
                    _round(nc, lambda q, g: psP[32 * q:32 * q + 32, g, :],
                           lambda sl, g: tser[sl, g, :],
                           lambda sl, g: tser[sl, g, :])
                    t2 = kpool.tile([128, GRP, 32], F16, tag="t2",
                                    name=f"t2_{b}_{c}")
                    nc.scalar.copy(t2[:], psP[:])
                    yield
                    psP = ps.tile([128, GRP, 32], F32, tag="pA", bufs=3,
                                  name=f"psT3_{b}_{c}")
                    _round(nc, lambda q, g: psP[32 * q:32 * q + 32, g, :],
                           lambda sl, g: tser[sl, g, :],
                           lambda sl, g: t2[sl, g, :])
                    t3 = kpool.tile([128, GRP, 32], F16, tag="t3",
                                    name=f"t3_{b}_{c}")
                    nc.scalar.copy(t3[:], psP[:])
                    qt = [kpool.tile([128, GRP, 32], F16, tag=f"q{i}",
                                     name=f"qt{i}_{b}_{c}")
                          for i in range(3)]
                    for i in range(3):
                        nc.vector.scalar_tensor_tensor(
                            qt[i][:], tser[:], float(MONO[3 * i + 1]),
                            ibser[:, i], op0=MULT, op1=ADD)
                        nc.vector.scalar_tensor_tensor(
                            qt[i][:], t2[:], float(MONO[3 * i + 2]), qt[i][:],
                            op0=MULT, op1=ADD)
                    h1 = kpool.tile([128, GRP, 32], F16, tag="h1",
                                    name=f"h1_{b}_{c}")
                    nc.vector.scalar_tensor_tensor(
                        h1[:], t3[:], float(c9), qt[2][:], op0=MULT, op1=ADD)
                    yield
                    psP = ps.tile([128, GRP, 32], F32, tag="pA", bufs=3,
                                  name=f"psH_{b}_{c}")
                    _round(nc, lambda q, g: psP[32 * q:32 * q + 32, g, :],
                           lambda sl, g: h1[sl, g, :],
                           lambda sl, g: t3[sl, g, :])
                    acc1 = kpool.tile([128, GRP, 32], F16, tag="acc1",
                                      name=f"ac_{b}_{c}")
                    nc.vector.tensor_tensor(acc1[:], psP[:], qt[1][:], op=ADD)
                    yield
                    # final Horner round + q0 add, output to strips 0-31
                    for half in range(2):
                        psL = ps.tile([32, 2, GRP, 32], F32, tag="pB", bufs=2,
                                      name=f"psL{b}_{c}_{half}")
                        # q0 first (start=True sets has_written across the
                        # region), then Horner matmuls accumulate onto it.
                        for ql in range(2):
                            q = 2 * half + ql
                            sl = slice(32 * q, 32 * q + 32)
                            nc.tensor.matmul(
                                psL[:, ql, :, :], ident32[sl, :],
                                qt[0][sl, :, :], start=True, stop=False,
                                tile_position=(32 * q, 0),
                                skip_group_check=True)
                        for g in range(GRP):
                            for ql in range(2):
                                q = 2 * half + ql
                                sl = slice(32 * q, 32 * q + 32)
                                nc.tensor.matmul(
                                    psL[:, ql, g, :], acc1[sl, g, :],
                                    t3[sl, g, :], start=False,
                                    stop=(g == GRP - 1),
                                    tile_position=(32 * q, 0),
                                    skip_group_check=True)
                        dst = lf[:, w, poff:poff + 64, :].rearrange(
                            "m (g f) j -> m f g j", f=4)[:, 2 * half:2 * half + 2]
                        nc.scalar.mul(dst, psL[:], 4.0)
                        yield

                for cp in range(0, NCHUNK, 2):
                    active = [chunk_gen(cp), chunk_gen(cp + 1)]
                    while active:
                        for g in list(active):
                            try:
                                next(g)
                            except StopIteration:
                                active.remove(g)

                if debug_stage <= 2:
                    continue
                if "d_lf" in dbg:
                    lff = bpool.tile([32, 3, P, 32], F32, tag="lff", bufs=1)
                    nc.vector.tensor_copy(lff[:], lf[:])
                    nc.sync.dma_start(dbg["d_lf"][b], lff[:])
                if debug_stage <= 3:
                    continue

                # ================= stage C: attention =================
                qrow = bpool.tile([1, 128], F32, tag="qrow", bufs=1)
                krow = bpool.tile([1, 128], F32, tag="krow", bufs=1)
                for kind, row in ((0, qrow), (1, krow)):
                    sqf = bpool.tile([32, P, 32], F16, tag="xt", bufs=1)
                    nc.scalar.activation(sqf[:], lf[:, kind],
                                         mybir.ActivationFunctionType.Square)
                    rsf = bpool.tile([32, P], F32, tag="rsf", bufs=1)
                    nc.vector.tensor_reduce(rsf[:], sqf[:],
                                            axis=mybir.AxisListType.X, op=ADD)
                    psq = ps.tile([1, 128], F32, tag="pA", bufs=3,
                                  name=f"psq{b}_{kind}")
                    nc.tensor.matmul(psq[:], onesc[0:32, 0:1], rsf[:],
                                     start=True, stop=True)
                    nc.scalar.mul(row[:], psq[:], -0.5)
                psE = ps.tile([128, 128], F32, tag="pB", bufs=2, name=f"psE{b}")
                for j in range(32):
                    nc.tensor.matmul(psE[:], lf[:, 1, :, j], lf[:, 0, :, j],
                                     start=(j == 0), stop=False)
                nc.tensor.matmul(psE[:], onesc[0:1, :], qrow[:],
                                 start=False, stop=False)
                nc.tensor.matmul(psE[:], krow[:], onesc[0:1, :],
                                 start=False, stop=True)
                w1 = bpool.tile([128, 128], F32, tag="w1", bufs=1)
                nc.scalar.activation(w1[:], psE[:],
                                     mybir.ActivationFunctionType.Relu,
                                     scale=-2.0)
                w2 = bpool.tile([128, 128], F32, tag="w2", bufs=1)
                nc.scalar.activation(w2[:], w1[:],
                                     mybir.ActivationFunctionType.Ln, bias=1.0)
                nc.vector.tensor_scalar_add(w2[:], w2[:], 1.0)
                wr = bpool.tile([128, 128], F32, tag="wr", bufs=1)
                nc.vector.reciprocal(wr[:], w2[:])
                srow = bpool.tile([128, 1], F32, tag="srow", bufs=1)
                ew = bpool.tile([128, 128], F32, tag="ew", bufs=1)
                nc.scalar.activation(ew[:], wr[:],
                                     mybir.ActivationFunctionType.Exp,
                                     accum_out=srow[:])
                rsrow = bpool.tile([128, 1], F32, tag="rsrow", bufs=1)
                nc.vector.reciprocal(rsrow[:], srow[:])
                stile = bpool.tile([128, 128], F32, tag="stile", bufs=1)
                nc.scalar.mul(stile[:], ew[:], rsrow[:])
                if "d_s" in dbg:
                    nc.sync.dma_start(dbg["d_s"][b], stile[:])
                psST = ps.tile([128, 128], F32, tag="pA", bufs=3, name=f"psST{b}")
                nc.tensor.transpose(psST[:], stile[:], id128[:])
                st_t = bpool.tile([128, 128], F16, tag="st_t", bufs=1)
                nc.scalar.copy(st_t[:], psST[:])
                lvfs = bpool.tile([128, 1024], F16, tag="lvfs", bufs=1)
                nc.sync.dma_start(scrV[:], lf[:, 2])
                nc.sync.dma_start(
                    lvfs[:].rearrange("m (i j) -> m i j", i=32),
                    scrV.rearrange("i m j -> m i j"))
                psML = ps.tile([128, 8, 128], F32, tag="pB", bufs=2, name=f"psML{b}")
                for cc in range(8):
                    nc.tensor.matmul(psML[:, cc, :], lvfs[:, 128 * cc:128 * (cc + 1)],
                                     st_t[:], start=True, stop=True)
                mlT = bpool.tile([128, 8, 128], F32, tag="mlT", bufs=1)
                nc.scalar.copy(mlT[:], psML[:])
                psMT = ps.tile([128, 8, 128], F32, tag="pB", bufs=2, name=f"psMT{b}")
                for cc in range(8):
                    nc.tensor.transpose(psMT[:, cc, :], mlT[:, cc, :], id128[:])
                mlfs = bpool.tile([128, 1024], F16, tag="mlfs", bufs=1)
                nc.scalar.mul(mlfs[:], psMT[:].rearrange("m c e -> m (c e)"),
                              1.0 / (2.0 ** EXP_K))
                if "d_mlfs" in dbg:
                    mlfs_f = bpool.tile([128, 1024], F32, tag="mlfsf", bufs=1)
                    nc.vector.tensor_copy(mlfs_f[:], mlfs[:])
                    nc.sync.dma_start(dbg["d_mlfs"][b], mlfs_f[:])
                if debug_stage <= 4:
                    continue

                # ================= stage D: expm =================
                gt = bpool.tile([128, 32, 32], F16, tag="gt", bufs=1)
                nc.sync.dma_start(scrM[:], mlfs[:])
                for rr in range(4):
                    nc.sync.dma_start(
                        gt[32 * rr:32 * rr + 32, :, :],
                        scrM[rr::4, :].rearrange("g (i j) -> i g j", i=32))
                if "d_gt" in dbg:
                    gt_f = bpool.tile([128, 32, 32], F32, tag="gtf", bufs=1)
                    nc.vector.tensor_copy(gt_f[:], gt[:])
                    nc.sync.dma_start(dbg["d_gt"][b], gt_f[:])
                if debug_stage == 5:
                    continue
                psX = ps.tile([128, 32, 32], F32, tag="pB", bufs=2, name=f"psA2{b}")
                _round(nc, lambda q, g: psX[32 * q:32 * q + 32, g, :],
                       lambda sl, g: gt[sl, g, :],
                       lambda sl, g: gt[sl, g, :], ngrp=32)
                a2 = bpool.tile([128, 32, 32], F16, tag="a2", bufs=1)
                nc.scalar.copy(a2[:], psX[:])
                psX = ps.tile([128, 32, 32], F32, tag="pB", bufs=2, name=f"psA3{b}")
                _round(nc, lambda q, g: psX[32 * q:32 * q + 32, g, :],
                       lambda sl, g: gt[sl, g, :],
                       lambda sl, g: a2[sl, g, :], ngrp=32)
                a3 = bpool.tile([128, 32, 32], F16, tag="a3", bufs=1)
                nc.scalar.copy(a3[:], psX[:])
                # q_i = ibexp[i] + c_{3i+1} A + c_{3i+2} A2  (Taylor, PS)
                qe = [bpool.tile([128, 32, 32], F16, tag=f"qe{i}", bufs=1,
                                 name=f"qe{i}_{b}")
                      for i in range(3)]
                for i in range(3):
                    nc.vector.scalar_tensor_tensor(
                        qe[i][:], gt[:], float(EXP_C[3 * i + 1]), ibexp[:, i],
                        op0=MULT, op1=ADD)
                    nc.vector.scalar_tensor_tensor(
                        qe[i][:], a2[:], float(EXP_C[3 * i + 2]), qe[i][:],
                        op0=MULT, op1=ADD)
                psX = ps.tile([128, 32, 32], F32, tag="pB", bufs=2, name=f"psH1{b}")
                _round(nc, lambda q, g: psX[32 * q:32 * q + 32, g, :],
                       lambda sl, g: qe[2][sl, g, :],
                       lambda sl, g: a3[sl, g, :], ngrp=32)
                eacc = bpool.tile([128, 32, 32], F16, tag="eacc", bufs=1)
                nc.vector.tensor_tensor(eacc[:], psX[:], qe[1][:], op=ADD)
                psX = ps.tile([128, 32, 32], F32, tag="pB", bufs=2, name=f"psH2{b}")
                _round(nc, lambda q, g: psX[32 * q:32 * q + 32, g, :],
                       lambda sl, g: eacc[sl, g, :],
                       lambda sl, g: a3[sl, g, :], ngrp=32)
                sq0 = bpool.tile([128, 32, 32], F32, tag="sq0", bufs=1)
                nc.vector.tensor_tensor(sq0[:], psX[:], qe[0][:], op=ADD)
                # squarings in f32
                for sq_i in range(EXP_K):
                    psX = ps.tile([128, 32, 32], F32, tag="pB", bufs=2,
                                  name=f"psS{b}_{sq_i}")
                    _round(nc, lambda q, g: psX[32 * q:32 * q + 32, g, :],
                           lambda sl, g: sq0[sl, g, :],
                           lambda sl, g: sq0[sl, g, :], ngrp=32)
                    if sq_i < EXP_K - 1:
                        nc.scalar.copy(sq0[:], psX[:])
                    else:
                        nc.vector.tensor_copy(sq0[:], psX[:])
                for rr in range(4):
                    nc.sync.dma_start(
                        out_d[b][rr::4].rearrange("g i j -> i g j"),
                        sq0[32 * rr:32 * rr + 32, :, :])
    nc.compile()
    return nc, dbg


def build_null():
    """Same dram-tensor IO as build_nc but near-zero compute, for timing
    calibration (isolates host/transfer overhead from kernel compute)."""
    nc = bacc.Bacc("TRN2", target_bir_lowering=False, debug=False,
                   num_devices=NCORES)
    nc.dram_tensor("x", [BLOC, P, IN, IN], F32, kind="ExternalInput").ap()
    nc.dram_tensor("wallT", [IN, 96], F32, kind="ExternalInput").ap()
    nc.dram_tensor("wall2", [96, 64], F32, kind="ExternalInput").ap()
    ibig_in = nc.dram_tensor("ibig", [128, GRP, 32], F32,
                             kind="ExternalInput").ap()
    nc.dram_tensor("ibser", [128, 4, GRP, 32], F16, kind="ExternalInput").ap()
    nc.dram_tensor("ident32", [128, 32], F16, kind="ExternalInput").ap()
    nc.dram_tensor("ibexp", [128, 3, 32, 32], F16, kind="ExternalInput").ap()
    nc.dram_tensor("id128", [128, 128], F32, kind="ExternalInput").ap()
    nc.dram_tensor("onesc", [96, 128], F32, kind="ExternalInput").ap()
    out_d = nc.dram_tensor("out", [BLOC, P, 32, 32], F32,
                           kind="ExternalOutput").ap()
    with tile.TileContext(nc) as tc:
        with tc.tile_pool(name="p", bufs=1) as pool:
            t = pool.tile([128, GRP, 32], F32)
            nc.sync.dma_start(t[:], ibig_in[:])
            for b in range(BLOC):
                nc.sync.dma_start(
                    out_d[b].rearrange("p i j -> p (i j)")[0:128, 0:512],
                    t[:].rearrange("p g j -> p (g j)"))
    nc.compile()
    return nc


def host_constants(Wq, Wk, Wv):
    wallT = np.concatenate([Wq.T, Wk.T, Wv.T], axis=1).astype(np.float32)
    def w2(W):
        WT = np.ascontiguousarray(W.T.astype(np.float32))
        return np.concatenate([WT[0:32], WT[32:64]], axis=1)
    wall2 = np.concatenate([w2(Wq), w2(Wk), w2(Wv)], axis=0)
    eye = np.eye(32, dtype=np.float32)
    ibig = np.broadcast_to(eye[None, :, None, :],
                           (4, 32, GRP, 32)).reshape(128, GRP, 32).copy()
    # ibser[:, i] = s_i * I for i in 0..2 (series q_i consts), 3 = -SM/SH
    scal = [MONO[0] + LNC / 4.0, MONO[3], MONO[6], -SM / SH]
    ibser = np.stack([s * ibig for s in scal], axis=1).astype(np.float16)
    ident32 = np.broadcast_to(eye[None], (4, 32, 32)).reshape(128, 32).astype(np.float16)
    ibgx = np.broadcast_to(eye[None, :, None, :],
                           (4, 32, 32, 32)).reshape(128, 32, 32).copy()
    escal = [EXP_C[0], EXP_C[3], EXP_C[6]]
    ibexp = np.stack([s * ibgx for s in escal], axis=1).astype(np.float16)
    id128 = np.eye(128, dtype=np.float32)
    onesc = np.ones((96, 128), dtype=np.float32)
    return {"wallT": wallT, "wall2": wall2, "ibig": ibig, "ibser": ibser,
            "ident32": ident32, "ibexp": ibexp, "id128": id128,
            "onesc": onesc}


_NC_CACHE = {}

def make_in_maps(x, Wq, Wk, Wv):
    consts = host_constants(np.asarray(Wq), np.asarray(Wk), np.asarray(Wv))
    x = np.asarray(x, dtype=np.float32)
    in_maps = []
    for c in range(NCORES):
        m = {"x": np.ascontiguousarray(x[BLOC * c:BLOC * (c + 1)])}
        m.update(consts)
        in_maps.append(m)
    return in_maps


def kernel(x, Wq, Wk, Wv):
    if "full" not in _NC_CACHE:
        _NC_CACHE["full"] = build_nc(99)
    nc, _ = _NC_CACHE["full"]
    in_maps = make_in_maps(x, Wq, Wk, Wv)
    res = run_bass_kernel_spmd(nc, in_maps, list(range(NCORES)))
    out = np.concatenate([res.results[c]["out"] for c in range(NCORES)], axis=0)
    return out.astype(np.float32)
